# revision 28
# baseline (speedup 1.0000x reference)
"""Trainium2 Bass kernel for nn_Attention_68118181314928.

Reference computation (per batch b):
    ctx[h,s]  = sum_e W_k[h,e] * enc[b,s,e]
    wq[h]     = sum_e W_q[h,e] * dec[b,e]
    score[s]  = sum_h V[h] * tanh(ctx[h,s] + wq[h])
    alpha     = softmax(score)
    hidden[h] = sum_s ctx[h,s] * alpha[s]
              = sum_e W_k[h,e] * (sum_s enc[b,s,e] * alpha[s])   # no ctx rematerialization

Distribution: pure data-parallel over batch B=32 across 8 NeuronCores
(4 batches/core); W_k / W_q / V replicated.  No collectives.

Host-side prep (layout only): enc is pre-transposed to encT [b, E, S] and the
weights to W_kT/W_qT [E, H] so that the contraction dim E lands on SBUF
partitions with fully-contiguous DMA rows; this avoids any on-chip transposes.

On-chip per core/batch:
  - ctx tiles [128h x 512s] via f32r matmuls (full-rate PE), accumulated in PSUM
  - ScalarE computes tanh(ctx + wq[h]) PSUM->SBUF with per-partition bias
  - PE matvec with V as stationary accumulates score [1, 2048] in PSUM
  - softmax on-chip (VectorE reductions + ScalarE exp with fused sum)
  - encd[e] = sum_s alpha[s]*encT[e,s] via fused VectorE multiply-reduce
  - hidden = W_kT.T @ encd as a tiny PE matvec for all 4 batches at the end
"""

import os
import sys

import numpy as np

for _p in ("/opt/trn_rl_repo", "/root/.axon_site/_ro/trn_rl_repo"):
    if os.path.isdir(_p) and _p not in sys.path:
        sys.path.insert(0, _p)

import concourse.bass as bass  # noqa: E402
import concourse.tile as tile  # noqa: E402
from concourse import bacc, mybir  # noqa: E402
from concourse import bass_utils  # noqa: E402

B, S, E, H = 32, 2048, 1024, 1024
NCORES = 8
BPC = B // NCORES  # batches per core
F32 = mybir.dt.float32
F32R = mybir.dt.float32r
P = 128
EC = E // P  # e-chunks (contraction)
HC = H // P  # h-chunks
SN = 512  # matmul moving free-dim (one PSUM bank of f32)
ST = S // SN  # s-tiles

SOFTMAX_SHIFT = 80.0  # constant softmax shift; valid while |score| << 88+shift

AF = mybir.ActivationFunctionType
ALU = mybir.AluOpType
AX = mybir.AxisListType


def _r(ap):
    """View an f32 AP as float32r for full-rate PE matmuls."""
    return ap.bitcast(F32R)


def build_bass():
    # KSTAGE bisect: 1=through score, 2=+softmax, 3=+bcast+encd, 4=full (default)
    stage = int(os.environ.get("KSTAGE", "4"))
    nc = bacc.Bacc("TRN2", target_bir_lowering=False, debug=False, num_devices=NCORES)

    encT = nc.dram_tensor("encT", [BPC, E, S], F32R, kind="ExternalInput")
    decT = nc.dram_tensor("decT", [E, BPC], F32R, kind="ExternalInput")
    wkT = nc.dram_tensor("wkT", [E, H], F32R, kind="ExternalInput")
    wqT = nc.dram_tensor("wqT", [E, H], F32R, kind="ExternalInput")
    vcol = nc.dram_tensor("vcol", [P, HC], F32R, kind="ExternalInput")
    hid_out = nc.dram_tensor("hidden", [BPC, H], F32, kind="ExternalOutput")
    score_out = nc.dram_tensor("score", [BPC, S], F32, kind="ExternalOutput")
    # internal DRAM bounce used to broadcast alpha across partitions
    alpha_dram = nc.dram_tensor("alpha_bounce", [BPC, S], F32)
    debug = os.environ.get("KDEBUG") == "1"
    if debug:
        dbg_scr = nc.dram_tensor("dbg_scr", [P, S], F32, kind="ExternalOutput")
        dbg_ab = nc.dram_tensor("dbg_ab", [P, S], F32, kind="ExternalOutput")
        dbg_encd = nc.dram_tensor("dbg_encd", [P, EC * BPC], F32, kind="ExternalOutput")
        dbg_encdr = nc.dram_tensor("dbg_encdr", [P, EC * BPC], F32, kind="ExternalOutput")

    with tile.TileContext(nc) as tc:
        with tc.tile_pool(name="persist", bufs=1) as persist:
            # --- resident tensors ---
            wk_sb = persist.tile([P, EC, H], F32R)  # W_kT as [e_p, e_chunk, h]
            v_sb = persist.tile([P, HC], F32R)  # V chunks on partitions
            nc.sync.dma_start(out=v_sb, in_=vcol.ap())
            dec_sb = persist.tile([P, EC, BPC], F32R)  # dec as [e_p, e_chunk, b]
            nc.sync.dma_start(
                out=dec_sb, in_=decT.ap().rearrange("(c p) b -> p c b", p=P)
            )
            wq_sb = persist.tile([P, HC, BPC], F32)  # wq result [h_p, h_chunk, b]
            encd_sb = persist.tile([P, EC, BPC], F32R)  # encd [e_p, e_chunk, b]
            encd_f32 = persist.tile([P, EC, BPC], F32)  # f32 accumulator
            r_col = persist.tile([BPC, 1], F32)  # per-batch 1/Z on partition b
            neg_shift = persist.tile([1, 1], F32)
            nc.vector.memset(neg_shift, -SOFTMAX_SHIFT)
            ones_f = persist.tile([1, P], F32)
            nc.vector.memset(ones_f, 1.0)
            ones_r = persist.tile([1, P], F32R)
            nc.vector.tensor_copy(ones_r, ones_f)

            # --- main loop over this core's batches ---
            with (
                tc.tile_pool(name="enc", bufs=2 * EC) as enc_pool,
                tc.tile_pool(name="tanh", bufs=4) as tanh_pool,
                tc.tile_pool(name="small", bufs=1) as small,
                tc.tile_pool(name="bcast", bufs=1) as bcast,
                tc.tile_pool(name="scratch", bufs=2) as scratch,
                tc.tile_pool(name="eparts", bufs=2) as eparts_pool,
                tc.tile_pool(name="dram_b", bufs=4, space="DRAM") as dram_pool,
                tc.tile_pool(name="psum_ctx", bufs=3, space="PSUM") as psum_ctx,
                tc.tile_pool(name="psum_score", bufs=2, space="PSUM") as psum_score,
                tc.tile_pool(name="psum_ab", bufs=1, space="PSUM") as psum_ab,
                tc.tile_pool(name="outp", bufs=1) as outp,
            ):
                # h-major wq preamble: per-h 0.5MB columns of W_qT/W_kT so
                # the PE and tanh unblock after ~1MB instead of 8MB
                wk_src = wkT.ap().rearrange("(c p) h -> p c h", p=P)
                wq_src = wqT.ap().rearrange("(c p) h -> p c h", p=P)
                with tc.tile_pool(name="wq_stream", bufs=3) as wq_stream:
                    with tc.tile_pool(name="psum_wq", bufs=2, space="PSUM") as psum_wq:
                        for h in range(HC):
                            hsl = slice(h * P, (h + 1) * P)
                            col = wq_stream.tile([P, EC, P], F32R, tag="wqcol")
                            nc.sync.dma_start(out=col, in_=wq_src[:, :, hsl])
                            nc.sync.dma_start(
                                out=wk_sb[:, :, hsl], in_=wk_src[:, :, hsl]
                            )
                            wq_ps = psum_wq.tile([P, BPC], F32, tag="wqps")
                            for e in range(EC):
                                nc.tensor.matmul(
                                    wq_ps,
                                    col[:, e, :],
                                    dec_sb[:, e, :],
                                    start=(e == 0),
                                    stop=(e == EC - 1),
                                )
                            nc.vector.tensor_copy(wq_sb[:, h, :], wq_ps)
                psum_hid = tc.alloc_tile_pool(name="psum_hid", bufs=1, space="PSUM")

                for b in range(BPC):
                    # encT[b] tiles: [128e x 2048s] per e-chunk
                    et = []
                    for e in range(EC):
                        t = enc_pool.tile([P, S], F32R, tag="et")
                        et.append(t)
                    # quarter-tile DMAs so the first s-tile's matmuls can
                    # start after ~2MB instead of the full 8MB batch load
                    for s in range(ST):
                        for e in range(EC):
                            ssl0 = slice(s * SN, (s + 1) * SN)
                            nc.sync.dma_start(
                                out=et[e][:, ssl0],
                                in_=encT.ap()[b, e * P : (e + 1) * P, ssl0],
                            )

                    score_sb = small.tile([1, S], F32, tag="score_sb")
                    # streaming softmax state: unnormalized weights use a
                    # constant shift (scores here are bounded ~|66|, so
                    # exp(score - 80) neither overflows nor kills precision);
                    # normalization by 1/Z happens once on the final hidden.
                    z_parts = small.tile([1, ST], F32, tag="zp")
                    eparts = eparts_pool.tile([P, EC, ST], F32, tag="ep")

                    for s in range(ST):
                        ssl = slice(s * SN, (s + 1) * SN)
                        sc_ps = psum_score.tile([1, SN], F32, tag="score_ps")
                        for h in range(HC):
                            ctx_ps = psum_ctx.tile([P, SN], F32, tag="ctx")
                            for e in range(EC):
                                nc.tensor.matmul(
                                    ctx_ps,
                                    wk_sb[:, e, h * P : (h + 1) * P],
                                    et[e][:, ssl],
                                    start=(e == 0),
                                    stop=(e == EC - 1),
                                )
                            th = tanh_pool.tile([P, SN], F32R, tag="th")
                            nc.scalar.activation(
                                th, ctx_ps, AF.Tanh, bias=wq_sb[:, h, b : b + 1]
                            )
                            nc.tensor.matmul(
                                sc_ps,
                                v_sb[:, h : h + 1],
                                th,
                                start=(h == 0),
                                stop=(h == HC - 1),
                            )
                        nc.scalar.copy(score_sb[:, ssl], sc_ps)

                        # streamed unnormalized softmax + weighted enc reduce
                        last = b == BPC - 1
                        ex = small.tile([1, SN], F32R if last else F32, tag="ex")
                        nc.scalar.activation(
                            ex,
                            sc_ps,
                            AF.Exp,
                            bias=neg_shift,
                            accum_out=z_parts[:, s : s + 1],
                        )
                        if last:
                            # PE is idle in the tail: broadcast via ones-matmul
                            ab = psum_ab.tile([P, SN], F32, tag="abp")
                            nc.tensor.matmul(ab, ones_r, ex)
                        else:
                            adr = dram_pool.tile([1, SN], F32, tag="adr")
                            nc.sync.dma_start(out=adr, in_=ex)
                            ab = bcast.tile([P, SN], F32, tag="ab")
                            nc.gpsimd.dma_start(out=ab, in_=adr.to_broadcast([P, SN]))
                        for e in range(EC):
                            scr = scratch.tile([P, SN], F32, tag="scr")
                            nc.vector.scalar_tensor_tensor(
                                out=scr,
                                in0=et[e][:, ssl],
                                scalar=1.0,
                                in1=ab,
                                op0=ALU.bypass,
                                op1=ALU.mult,
                                accum_out=eparts[:, e, s : s + 1],
                            )

                    # raw (pre-softmax) score is an output
                    nc.sync.dma_start(out=score_out.ap()[b : b + 1, :], in_=score_sb)

                    # fold s-tile partials per e-chunk: encd[:, e, b]
                    for e in range(EC):
                        nc.vector.tensor_reduce(
                            encd_f32[:, e, b : b + 1],
                            eparts[:, e, :],
                            axis=AX.X,
                            op=ALU.add,
                        )
                    zsum = small.tile([1, 1], F32, tag="z")
                    nc.vector.tensor_reduce(zsum, z_parts, axis=AX.X, op=ALU.add)
                    rz = small.tile([1, 1], F32, tag="rz")
                    nc.vector.reciprocal(rz, zsum)
                    nc.sync.dma_start(out=r_col[b : b + 1, :], in_=rz)

                # --- epilogue (same pools): hidden = encd.T @ W_kT, per-e so
                # each matmul overlaps the remaining encd chain of batch 3
                hid_ps = psum_hid.tile([BPC, H], F32, tag="hid")
                for e in range(EC):
                    nc.vector.tensor_copy(encd_sb[:, e, :], encd_f32[:, e, :])
                    for g in range(H // SN):
                        gsl = slice(g * SN, (g + 1) * SN)
                        nc.tensor.matmul(
                            hid_ps[:, gsl],
                            encd_sb[:, e, :],
                            wk_sb[:, e, gsl],
                            start=(e == 0),
                            stop=(e == EC - 1),
                        )
                hid_sb = outp.tile([BPC, H], F32, tag="hid_sb")
                nc.vector.tensor_scalar_mul(hid_sb, hid_ps, r_col)
                nc.sync.dma_start(out=hid_out.ap(), in_=hid_sb)
                psum_hid.release()

    nc.compile()
    return nc


def prep_in_maps(enc, dec, Wk, Wq, V):
    enc = np.ascontiguousarray(np.asarray(enc, dtype=np.float32))
    dec = np.ascontiguousarray(np.asarray(dec, dtype=np.float32))
    wkT = np.ascontiguousarray(np.asarray(Wk, dtype=np.float32).T)
    wqT = np.ascontiguousarray(np.asarray(Wq, dtype=np.float32).T)
    vcol = np.ascontiguousarray(
        np.asarray(V, dtype=np.float32).reshape(HC, P).T
    )
    in_maps = []
    for c in range(NCORES):
        sl = slice(c * BPC, (c + 1) * BPC)
        in_maps.append(
            {
                "encT": np.ascontiguousarray(enc[sl].transpose(0, 2, 1)),
                "decT": np.ascontiguousarray(dec[sl].T),
                "wkT": wkT,
                "wqT": wqT,
                "vcol": vcol,
            }
        )
    return in_maps


_NC_CACHE = None


def _get_nc():
    global _NC_CACHE
    if _NC_CACHE is None:
        _NC_CACHE = build_bass()
    return _NC_CACHE


def run(inputs, **spmd_kwargs):
    """Run the kernel; returns ((hidden, score), BassKernelResults)."""
    nc = _get_nc()
    in_maps = prep_in_maps(
        inputs["enc_hidden"],
        inputs["dec_hidden"],
        inputs["W_k"],
        inputs["W_q"],
        inputs["V"],
    )
    res = bass_utils.run_bass_kernel_spmd(
        nc, in_maps, core_ids=list(range(NCORES)), **spmd_kwargs
    )
    hidden = np.concatenate([res.results[c]["hidden"] for c in range(NCORES)], axis=0)
    score = np.concatenate([res.results[c]["score"] for c in range(NCORES)], axis=0)
    return (hidden.astype(np.float32), score.astype(np.float32)), res


def kernel(enc_hidden, dec_hidden, W_k, W_q, V):
    outs, _ = run(
        {
            "enc_hidden": enc_hidden,
            "dec_hidden": dec_hidden,
            "W_k": W_k,
            "W_q": W_q,
            "V": V,
        }
    )
    return outs


# revision 30
# speedup vs baseline: 1.1629x; 1.1629x over previous
"""Trainium2 Bass kernel for nn_Attention_68118181314928.

Reference computation (per batch b):
    ctx[h,s]  = sum_e W_k[h,e] * enc[b,s,e]
    wq[h]     = sum_e W_q[h,e] * dec[b,e]
    score[s]  = sum_h V[h] * tanh(ctx[h,s] + wq[h])
    alpha     = softmax(score)
    hidden[h] = sum_s ctx[h,s] * alpha[s]
              = sum_e W_k[h,e] * (sum_s enc[b,s,e] * alpha[s])   # no ctx rematerialization

Distribution: pure data-parallel over batch B=32 across 8 NeuronCores
(4 batches/core); W_k / W_q / V replicated.  No collectives.

Host-side prep (layout only): enc is pre-transposed to encT [b, E, S] and the
weights to W_kT/W_qT [E, H] so that the contraction dim E lands on SBUF
partitions with fully-contiguous DMA rows; this avoids any on-chip transposes.

On-chip per core/batch:
  - ctx tiles [128h x 512s] via f32r matmuls (full-rate PE), accumulated in PSUM
  - ScalarE computes tanh(ctx + wq[h]) PSUM->SBUF with per-partition bias
  - PE matvec with V as stationary accumulates score [1, 2048] in PSUM
  - softmax on-chip (VectorE reductions + ScalarE exp with fused sum)
  - encd[e] = sum_s alpha[s]*encT[e,s] via fused VectorE multiply-reduce
  - hidden = W_kT.T @ encd as a tiny PE matvec for all 4 batches at the end
"""

import os
import sys

import numpy as np

for _p in ("/opt/trn_rl_repo", "/root/.axon_site/_ro/trn_rl_repo"):
    if os.path.isdir(_p) and _p not in sys.path:
        sys.path.insert(0, _p)

import concourse.bass as bass  # noqa: E402
import concourse.tile as tile  # noqa: E402
from concourse import bacc, mybir  # noqa: E402
from concourse import bass_utils  # noqa: E402

B, S, E, H = 32, 2048, 1024, 1024
NCORES = 8
BPC = B // NCORES  # batches per core
F32 = mybir.dt.float32
F32R = mybir.dt.float32r
P = 128
EC = E // P  # e-chunks (contraction)
HC = H // P  # h-chunks
SN = 512  # matmul moving free-dim (one PSUM bank of f32)
ST = S // SN  # s-tiles

SOFTMAX_SHIFT = 80.0  # constant softmax shift; valid while |score| << 88+shift

AF = mybir.ActivationFunctionType
ALU = mybir.AluOpType
AX = mybir.AxisListType


def _r(ap):
    """View an f32 AP as float32r for full-rate PE matmuls."""
    return ap.bitcast(F32R)


def build_bass():
    # KSTAGE bisect: 1=through score, 2=+softmax, 3=+bcast+encd, 4=full (default)
    stage = int(os.environ.get("KSTAGE", "4"))
    nc = bacc.Bacc("TRN2", target_bir_lowering=False, debug=False, num_devices=NCORES)

    encT = nc.dram_tensor("encT", [BPC, E, S], F32R, kind="ExternalInput")
    decT = nc.dram_tensor("decT", [E, BPC], F32R, kind="ExternalInput")
    wkT = nc.dram_tensor("wkT", [E, H], F32R, kind="ExternalInput")
    wqT = nc.dram_tensor("wqT", [E, H], F32R, kind="ExternalInput")
    vcol = nc.dram_tensor("vcol", [P, HC], F32R, kind="ExternalInput")
    hid_out = nc.dram_tensor("hidden", [BPC, H], F32, kind="ExternalOutput")
    score_out = nc.dram_tensor("score", [BPC, S], F32, kind="ExternalOutput")
    # internal DRAM bounce used to broadcast alpha across partitions
    alpha_dram = nc.dram_tensor("alpha_bounce", [BPC, S], F32)
    debug = os.environ.get("KDEBUG") == "1"
    if debug:
        dbg_scr = nc.dram_tensor("dbg_scr", [P, S], F32, kind="ExternalOutput")
        dbg_ab = nc.dram_tensor("dbg_ab", [P, S], F32, kind="ExternalOutput")
        dbg_encd = nc.dram_tensor("dbg_encd", [P, EC * BPC], F32, kind="ExternalOutput")
        dbg_encdr = nc.dram_tensor("dbg_encdr", [P, EC * BPC], F32, kind="ExternalOutput")

    with tile.TileContext(nc) as tc:
        with tc.tile_pool(name="persist", bufs=1) as persist:
            # --- resident tensors ---
            wk_sb = persist.tile([P, EC, H], F32R)  # W_kT as [e_p, e_chunk, h]
            v_sb = persist.tile([P, HC], F32R)  # V chunks on partitions
            nc.sync.dma_start(out=v_sb, in_=vcol.ap())
            dec_sb = persist.tile([P, EC, BPC], F32R)  # dec as [e_p, e_chunk, b]
            nc.sync.dma_start(
                out=dec_sb, in_=decT.ap().rearrange("(c p) b -> p c b", p=P)
            )
            wq_sb = persist.tile([P, HC, BPC], F32)  # wq result [h_p, h_chunk, b]
            encd_sb = persist.tile([P, EC, BPC], F32R)  # encd [e_p, e_chunk, b]
            encd_f32 = persist.tile([P, EC, BPC], F32)  # f32 accumulator
            r_col = persist.tile([BPC, 1], F32)  # per-batch 1/Z on partition b
            neg_shift = persist.tile([1, 1], F32)
            nc.vector.memset(neg_shift, -SOFTMAX_SHIFT)
            ones_f = persist.tile([1, P], F32)
            nc.vector.memset(ones_f, 1.0)
            ones_r = persist.tile([1, P], F32R)
            nc.vector.tensor_copy(ones_r, ones_f)

            # --- preamble: wq[h, b] = sum_e W_q[h, e] dec[b, e] ---
            with tc.tile_pool(name="wq_stream", bufs=2) as wq_stream:
                with tc.tile_pool(name="psum_wq", bufs=1, space="PSUM") as psum_wq:
                    wq_ps = [
                        psum_wq.tile([P, BPC], F32, tag=f"wq{h}", name=f"wq_ps{h}")
                        for h in range(HC)
                    ]
                    for e in range(EC):
                        ch = wq_stream.tile([P, H], F32R, tag="wqT")
                        nc.sync.dma_start(out=ch, in_=wqT.ap()[e * P : (e + 1) * P, :])
                        for h in range(HC):
                            nc.tensor.matmul(
                                wq_ps[h],
                                ch[:, h * P : (h + 1) * P],
                                dec_sb[:, e, :],
                                start=(e == 0),
                                stop=(e == EC - 1),
                            )
                    for h in range(HC):
                        nc.vector.tensor_copy(wq_sb[:, h, :], wq_ps[h])

            # W_k lands after the wq stream: h0 column sliced out so the
            # first ctx matmul unblocks early, the rest as one bulk DMA with
            # 3.5KB contiguous rows (512B-row column DMAs are inefficient)
            wk_src = wkT.ap().rearrange("(c p) h -> p c h", p=P)
            nc.sync.dma_start(out=wk_sb[:, :, :P], in_=wk_src[:, :, :P])
            nc.sync.dma_start(out=wk_sb[:, :, P:], in_=wk_src[:, :, P:])

            # --- main loop over this core's batches ---
            with (
                tc.tile_pool(name="enc", bufs=2 * EC) as enc_pool,
                tc.tile_pool(name="tanh", bufs=4) as tanh_pool,
                tc.tile_pool(name="small", bufs=1) as small,
                tc.tile_pool(name="bcast", bufs=1) as bcast,
                tc.tile_pool(name="scratch", bufs=2) as scratch,
                tc.tile_pool(name="eparts", bufs=2) as eparts_pool,
                tc.tile_pool(name="dram_b", bufs=4, space="DRAM") as dram_pool,
                tc.tile_pool(name="psum_ctx", bufs=3, space="PSUM") as psum_ctx,
                tc.tile_pool(name="psum_score", bufs=2, space="PSUM") as psum_score,
                tc.tile_pool(name="psum_ab", bufs=1, space="PSUM") as psum_ab,
                tc.tile_pool(name="psum_hid", bufs=1, space="PSUM") as psum_hid,
                tc.tile_pool(name="outp", bufs=1) as outp,
            ):
                for b in range(BPC):
                    # encT[b] tiles: [128e x 2048s] per e-chunk
                    et = []
                    for e in range(EC):
                        t = enc_pool.tile([P, S], F32R, tag="et")
                        et.append(t)
                    # quarter-tile DMAs so the first s-tile's matmuls can
                    # start after ~2MB instead of the full 8MB batch load
                    for s in range(ST):
                        for e in range(EC):
                            ssl0 = slice(s * SN, (s + 1) * SN)
                            nc.sync.dma_start(
                                out=et[e][:, ssl0],
                                in_=encT.ap()[b, e * P : (e + 1) * P, ssl0],
                            )

                    score_sb = small.tile([1, S], F32, tag="score_sb")
                    # streaming softmax state: unnormalized weights use a
                    # constant shift (scores here are bounded ~|66|, so
                    # exp(score - 80) neither overflows nor kills precision);
                    # normalization by 1/Z happens once on the final hidden.
                    z_parts = small.tile([1, ST], F32, tag="zp")
                    eparts = eparts_pool.tile([P, EC, ST], F32, tag="ep")

                    for s in range(ST):
                        ssl = slice(s * SN, (s + 1) * SN)
                        sc_ps = psum_score.tile([1, SN], F32, tag="score_ps")
                        for h in range(HC):
                            ctx_ps = psum_ctx.tile([P, SN], F32, tag="ctx")
                            for e in range(EC):
                                nc.tensor.matmul(
                                    ctx_ps,
                                    wk_sb[:, e, h * P : (h + 1) * P],
                                    et[e][:, ssl],
                                    start=(e == 0),
                                    stop=(e == EC - 1),
                                )
                            th = tanh_pool.tile([P, SN], F32R, tag="th")
                            nc.scalar.activation(
                                th, ctx_ps, AF.Tanh, bias=wq_sb[:, h, b : b + 1]
                            )
                            nc.tensor.matmul(
                                sc_ps,
                                v_sb[:, h : h + 1],
                                th,
                                start=(h == 0),
                                stop=(h == HC - 1),
                            )
                        nc.scalar.copy(score_sb[:, ssl], sc_ps)

                        # streamed unnormalized softmax + weighted enc reduce
                        last = b == BPC - 1
                        ex = small.tile([1, SN], F32R if last else F32, tag="ex")
                        nc.scalar.activation(
                            ex,
                            sc_ps,
                            AF.Exp,
                            bias=neg_shift,
                            accum_out=z_parts[:, s : s + 1],
                        )
                        if last:
                            # PE is idle in the tail: broadcast via ones-matmul
                            ab = psum_ab.tile([P, SN], F32, tag="abp")
                            nc.tensor.matmul(ab, ones_r, ex)
                        else:
                            adr = dram_pool.tile([1, SN], F32, tag="adr")
                            nc.sync.dma_start(out=adr, in_=ex)
                            ab = bcast.tile([P, SN], F32, tag="ab")
                            nc.gpsimd.dma_start(out=ab, in_=adr.to_broadcast([P, SN]))
                        for e in range(EC):
                            scr = scratch.tile([P, SN], F32, tag="scr")
                            nc.vector.scalar_tensor_tensor(
                                out=scr,
                                in0=et[e][:, ssl],
                                scalar=1.0,
                                in1=ab,
                                op0=ALU.bypass,
                                op1=ALU.mult,
                                accum_out=eparts[:, e, s : s + 1],
                            )

                    # raw (pre-softmax) score is an output
                    nc.sync.dma_start(out=score_out.ap()[b : b + 1, :], in_=score_sb)

                    # fold s-tile partials per e-chunk: encd[:, e, b]
                    for e in range(EC):
                        nc.vector.tensor_reduce(
                            encd_f32[:, e, b : b + 1],
                            eparts[:, e, :],
                            axis=AX.X,
                            op=ALU.add,
                        )
                    zsum = small.tile([1, 1], F32, tag="z")
                    nc.vector.tensor_reduce(zsum, z_parts, axis=AX.X, op=ALU.add)
                    rz = small.tile([1, 1], F32, tag="rz")
                    nc.vector.reciprocal(rz, zsum)
                    nc.sync.dma_start(out=r_col[b : b + 1, :], in_=rz)

                # --- epilogue (same pools): hidden = encd.T @ W_kT, per-e so
                # each matmul overlaps the remaining encd chain of batch 3
                hid_ps = psum_hid.tile([BPC, H], F32, tag="hid")
                for e in range(EC):
                    nc.vector.tensor_copy(encd_sb[:, e, :], encd_f32[:, e, :])
                    for g in range(H // SN):
                        gsl = slice(g * SN, (g + 1) * SN)
                        nc.tensor.matmul(
                            hid_ps[:, gsl],
                            encd_sb[:, e, :],
                            wk_sb[:, e, gsl],
                            start=(e == 0),
                            stop=(e == EC - 1),
                        )
                hid_sb = outp.tile([BPC, H], F32, tag="hid_sb")
                nc.vector.tensor_scalar_mul(hid_sb, hid_ps, r_col)
                nc.sync.dma_start(out=hid_out.ap(), in_=hid_sb)

    nc.compile()
    return nc


def prep_in_maps(enc, dec, Wk, Wq, V):
    enc = np.ascontiguousarray(np.asarray(enc, dtype=np.float32))
    dec = np.ascontiguousarray(np.asarray(dec, dtype=np.float32))
    wkT = np.ascontiguousarray(np.asarray(Wk, dtype=np.float32).T)
    wqT = np.ascontiguousarray(np.asarray(Wq, dtype=np.float32).T)
    vcol = np.ascontiguousarray(
        np.asarray(V, dtype=np.float32).reshape(HC, P).T
    )
    in_maps = []
    for c in range(NCORES):
        sl = slice(c * BPC, (c + 1) * BPC)
        in_maps.append(
            {
                "encT": np.ascontiguousarray(enc[sl].transpose(0, 2, 1)),
                "decT": np.ascontiguousarray(dec[sl].T),
                "wkT": wkT,
                "wqT": wqT,
                "vcol": vcol,
            }
        )
    return in_maps


_NC_CACHE = None


def _get_nc():
    global _NC_CACHE
    if _NC_CACHE is None:
        _NC_CACHE = build_bass()
    return _NC_CACHE


def run(inputs, **spmd_kwargs):
    """Run the kernel; returns ((hidden, score), BassKernelResults)."""
    nc = _get_nc()
    in_maps = prep_in_maps(
        inputs["enc_hidden"],
        inputs["dec_hidden"],
        inputs["W_k"],
        inputs["W_q"],
        inputs["V"],
    )
    res = bass_utils.run_bass_kernel_spmd(
        nc, in_maps, core_ids=list(range(NCORES)), **spmd_kwargs
    )
    hidden = np.concatenate([res.results[c]["hidden"] for c in range(NCORES)], axis=0)
    score = np.concatenate([res.results[c]["score"] for c in range(NCORES)], axis=0)
    return (hidden.astype(np.float32), score.astype(np.float32)), res


def kernel(enc_hidden, dec_hidden, W_k, W_q, V):
    outs, _ = run(
        {
            "enc_hidden": enc_hidden,
            "dec_hidden": dec_hidden,
            "W_k": W_k,
            "W_q": W_q,
            "V": V,
        }
    )
    return outs
